# revision 2
# baseline (speedup 1.0000x reference)
"""Bahdanau attention scores on 8 TRN2 NeuronCores (data-parallel, batch/8).

Reference:
    en  = tanh(einsum('sbf,df->sbd', concat([hid_bcast, enc], -1), W) + b)
    out = softmax(einsum('sbd,d->bs', en, v), axis=1)

Design (measured on HW via NTFF traces):
  * Transposed layout: s on PSUM partitions (128/s-tile), dec on the moving
    axis.  Stationary operand = x-tile [128 enc, 128 s]; moving = W_e column
    halves [128, 512].  The batch-independent W_e stays resident in SBUF.
  * fp16 operands: full PE rate (1 col/cycle) like f32r, but LDWEIGHTS rides
    FWL (116ns, fully hidden under the 213ns N=512 stream).  f32r weight
    loads (224ns fp32_mode=HIGH) cost +65ns/MM; walrus emits an LDW per
    matmul (no dedupe; --enable-ldw-opt crashes codegen).  fp16 keeps
    rel err ~1.7e-3 (bf16 was ~1.9e-2: 8 vs 10 mantissa bits).
  * h-major accumulation groups (all 8 k-matmuls per PSUM bank, then the
    next bank): PSUM write-bank switches cost ~46ns/MM when alternating.
    Steady-state MM issue gap: 216ns = the 512-cycle streaming floor.
  * v-weighted dec-reduction on the Vector engine: one scalar_tensor_tensor
    (en*v, accum_out) per half, no PE matmuls.  (tensor_tensor_reduce is a
    custom DVE op that wedges this runtime's exec unit - do not use.)
  * hid_proj entirely on-chip: hpT5[0:4] = hid.T @ W_h via 16 matmuls whose
    stationary is hidT [128, 4] (4-col weight loads), row 4 = attn_b; then
    per-batch select-matmuls (sel5[b] = e_b + e_bias, K=5) broadcast
    hp[b] + bias across all 128 partitions; ACT copies them to SBUF.
  * Softmax: Exp with accum_out, GpSimd partition_all_reduce (each partition
    gets the batch total), DVE reciprocal + scale, contiguous out DMA.
  * Startup: PE starts on (b0, st0/st1) immediately; the hp chain is
    interleaved between them.  Three DMA rings (sync: we+x stream,
    scalar: x_first+consts, gpsimd: W_h) cut the first-MM latency.

Host-side prep (free; graded time is the on-device NEFF span): per-core
shards prepacked so every DMA moves contiguous >=2KB per-partition lines;
output returns [b, s_tile, s_low] and is re-rolled on host.

Known run-to-run variance: the chip executes at 2.4GHz or 2.0GHz depending
on power state (everything scales 1.2x; nothing the kernel can control).
"""

import numpy as np

S = 2048
B = 32
E = 1024
D = 1024
N_CORES = 8
BL = B // N_CORES     # 4 local batches
ST = S // 128         # 16 s-tiles
KT = E // 128         # 8 enc contraction tiles
KH = D // 128         # 8 hid contraction tiles

_COMPILED = None
LAST_RESULTS = None
PROFILE = False
TRACE_KWARGS = {}


def _build():
    import concourse.bacc as bacc
    import concourse.mybir as mybir
    from concourse.tile import TileContext

    f32 = mybir.dt.float32
    bf16 = mybir.dt.bfloat16
    fp16 = mybir.dt.float16
    Tanh = mybir.ActivationFunctionType.Tanh
    Exp = mybir.ActivationFunctionType.Exp
    Mult = mybir.AluOpType.mult
    Add = mybir.AluOpType.add
    import concourse.bass_isa as bass_isa

    nc = bacc.Bacc("TRN2", target_bir_lowering=False, debug=False)

    xT = nc.dram_tensor("xT", [BL, ST, 128, KT, 128], fp16, kind="ExternalInput")
    weM = nc.dram_tensor("weM", [KT, 128, 2, 512], fp16, kind="ExternalInput")
    whM = nc.dram_tensor("whM", [KH, 128, 2, 512], fp16, kind="ExternalInput")
    hidT = nc.dram_tensor("hidT", [128, KH, BL], fp16, kind="ExternalInput")
    vB = nc.dram_tensor("vB", [128, 2, 512], fp16, kind="ExternalInput")
    attn_bT = nc.dram_tensor("attn_bT", [1, 2, 512], fp16, kind="ExternalInput")
    sel5 = nc.dram_tensor("sel5", [5, BL, 128], fp16, kind="ExternalInput")
    out = nc.dram_tensor("out", [BL, 128, ST], f32, kind="ExternalOutput")

    with TileContext(nc) as tc:
        with (
            tc.tile_pool(name="const", bufs=1) as constp,
            tc.tile_pool(name="xp", bufs=6) as xp,
            tc.tile_pool(name="work", bufs=2) as workp,
            tc.tile_pool(name="mmps", bufs=8, space="PSUM") as mmps,
        ):
            we_sb = constp.tile([128, KT, 2, 512], fp16)
            wh_sb = constp.tile([128, KH, 2, 512], fp16)
            hid_sb = constp.tile([128, KH, BL], fp16)
            v_sb = constp.tile([128, 2, 512], fp16)
            sel5_sb = constp.tile([5, BL, 128], fp16)
            hpT5_sb = constp.tile([5, 2, 512], fp16)
            hpb_sb = constp.tile([128, BL, 2, 512], f32)
            scores_sb = constp.tile([128, BL, ST], f32)
            exps_sb = constp.tile([128, BL, ST], f32)
            rs_sb = constp.tile([128, BL], f32)
            tot_sb = constp.tile([128, BL], f32)
            rcp_sb = constp.tile([128, BL], f32)
            out_sb = constp.tile([128, BL, ST], f32)

            # scalar ring: x_first + small consts; gpsimd ring: hp weights
            x_first = constp.tile([128, KT, 128], fp16)
            nc.scalar.dma_start(out=x_first[:, :, :], in_=xT[0, 0, :, :, :])
            nc.scalar.dma_start(out=hpT5_sb[4:5, :, :], in_=attn_bT[:, :, :])
            nc.scalar.dma_start(out=sel5_sb[:, :, :], in_=sel5[:, :, :])
            nc.gpsimd.dma_start(out=hid_sb[:, :, :], in_=hidT[:, :, :])
            for k in range(KH):
                nc.gpsimd.dma_start(out=wh_sb[:, k, :, :], in_=whM[k, :, :, :])
            # sync ring: we slabs, then x stream
            for k in range(KT):
                nc.sync.dma_start(out=we_sb[:, k, :, :], in_=weM[k, :, :, :])
            x_second = constp.tile([128, KT, 128], fp16)
            nc.sync.dma_start(out=x_second[:, :, :], in_=xT[0, 1, :, :, :])
            nc.sync.dma_start(out=v_sb[:, :, :], in_=vB[:, :, :])

            # hp chain, all on-chip: hpT5 rows 0..3 = hid.T @ Wh (psum ->
            # fp16 copies), row 4 = attn_b; then per-batch select-matmuls
            # (sel5[b] = e_b + e_bias) broadcast hp+bias across partitions
            def emit_hp():
                hp_ps = [mmps.tile([BL, 512], f32, name=f"hp_ps{h}", tag="mm")
                         for h in range(2)]
                for h in range(2):
                    for k in range(KH):
                        nc.tensor.matmul(
                            hp_ps[h][:, :],
                            lhsT=hid_sb[:, k, :],
                            rhs=wh_sb[:, k, h, :],
                            start=(k == 0),
                            stop=(k == KH - 1),
                        )
                for h in range(2):
                    nc.vector.tensor_copy(
                        out=hpT5_sb[0:BL, h, :], in_=hp_ps[h][:, :],
                    )

            def emit_hp_bcast():
                for b in range(BL):
                    for h in range(2):
                        bc = mmps.tile([128, 512], f32, name="bc", tag="mm")
                        nc.tensor.matmul(
                            bc[:, :],
                            lhsT=sel5_sb[:, b, :],
                            rhs=hpT5_sb[:, h, :],
                            start=True, stop=True,
                        )
                        nc.scalar.activation(
                            out=hpb_sb[:, b, h, :], in_=bc[:, :],
                            func=mybir.ActivationFunctionType.Copy,
                        )

            def emit_dve(b, st, mm):
                pre = workp.tile([128, 2, 512], f32, tag="pre")
                en = workp.tile([128, 2, 512], fp16, tag="en")
                ttr_out = workp.tile([128, 2, 512], fp16, tag="ttro")
                acc0 = workp.tile([128, 1], f32, tag="acc")
                for h in range(2):
                    nc.vector.tensor_tensor(
                        out=pre[:, h, :], in0=mm[h][:, :],
                        in1=hpb_sb[:, b, h, :], op=Add,
                    )
                    nc.scalar.activation(
                        out=en[:, h, :], in_=pre[:, h, :], func=Tanh,
                    )
                    # per-half v-reduce; h0 pass overlaps the h1 matmuls
                    nc.vector.scalar_tensor_tensor(
                        out=ttr_out[:, h, :], in0=en[:, h, :],
                        scalar=1.0, in1=v_sb[:, h, :], op0=Mult, op1=Mult,
                        accum_out=(acc0[:, :] if h == 0
                                   else scores_sb[:, b, st:st + 1]),
                    )
                nc.gpsimd.tensor_scalar_add(
                    out=scores_sb[:, b, st:st + 1],
                    in0=scores_sb[:, b, st:st + 1], scalar1=acc0[:, :],
                )

            deferred = []
            for b in range(BL):
                for st in range(ST):
                    if b == 0 and st == 0:
                        x_t = x_first
                    elif b == 0 and st == 1:
                        x_t = x_second
                    else:
                        x_t = xp.tile([128, KT, 128], fp16, tag="x")
                        nc.sync.dma_start(out=x_t[:, :, :], in_=xT[b, st, :, :, :])
                    mm = [mmps.tile([128, 512], f32, name=f"mm{h}", tag="mm")
                          for h in range(2)]
                    for h in range(2):
                        for k in range(KT):
                            nc.tensor.matmul(
                                mm[h][:, :],
                                lhsT=x_t[:, k, :],
                                rhs=we_sb[:, k, h, :],
                                start=(k == 0),
                                stop=(k == KT - 1),
                            )
                    if b == 0 and st < 2:
                        # PE gets going on st0 immediately; the hp chain is
                        # interleaved between the first s-tiles so hpb_sb is
                        # ready before the DVE backlog matters
                        deferred.append((b, st, mm))
                        if st == 0:
                            emit_hp()
                        if st == 1:
                            emit_hp_bcast()
                            for args in deferred:
                                emit_dve(*args)
                            deferred = None
                        continue
                    emit_dve(b, st, mm)
                # per-batch softmax (no PE involvement)
                nc.scalar.activation(
                    out=exps_sb[:, b, :], in_=scores_sb[:, b, :], func=Exp,
                    accum_out=rs_sb[:, b:b + 1],
                )
                nc.gpsimd.partition_all_reduce(
                    out_ap=tot_sb[:, b:b + 1], in_ap=rs_sb[:, b:b + 1],
                    channels=128, reduce_op=bass_isa.ReduceOp.add,
                )
                nc.vector.reciprocal(out=rcp_sb[:, b:b + 1], in_=tot_sb[:, b:b + 1])
                nc.vector.tensor_scalar_mul(
                    out=out_sb[:, b, :], in0=exps_sb[:, b, :],
                    scalar1=rcp_sb[:, b:b + 1],
                )
                nc.sync.dma_start(out=out[b, :, :], in_=out_sb[:, b, :])

    nc.compile()
    return nc


def _prep_in_maps(hidden, encoder_outputs, attn_w, attn_b, v):
    import ml_dtypes
    W_h = attn_w[:, :D]
    W_e = attn_w[:, D:]
    weM = np.ascontiguousarray(W_e.T.reshape(KT, 128, 2, 512)).astype(np.float16)
    whM = np.ascontiguousarray(W_h.T.reshape(KH, 128, 2, 512)).astype(np.float16)
    vB = np.ascontiguousarray(np.tile(v, (128, 1)).reshape(128, 2, 512)).astype(
        np.float16)
    attn_bT = np.ascontiguousarray(attn_b.reshape(1, 2, 512)).astype(np.float16)
    sel5 = np.zeros((5, BL, 128), dtype=np.float16)
    for b in range(BL):
        sel5[b, b, :] = 1.0
        sel5[4, b, :] = 1.0

    in_maps = []
    for c in range(N_CORES):
        b0 = c * BL
        # [b, st, p(enc), k, j(s)]
        xT = np.ascontiguousarray(
            encoder_outputs[:, b0:b0 + BL, :]        # [S, BL, E]
            .transpose(1, 0, 2)                      # [BL, S, E]
            .reshape(BL, ST, 128, KT, 128)           # [b, st, j, k, p]
            .transpose(0, 1, 4, 3, 2)).astype(np.float16)  # [b, st, p, k, j]
        hidT = np.ascontiguousarray(
            hidden[b0:b0 + BL, :].T.reshape(KH, 128, BL).transpose(1, 0, 2)
        ).astype(np.float16)
        in_maps.append({
            "xT": xT, "weM": weM, "whM": whM, "hidT": hidT,
            "vB": vB, "attn_bT": attn_bT, "sel5": sel5,
        })
    return in_maps


def kernel(hidden, encoder_outputs, attn_w, attn_b, v):
    global _COMPILED, LAST_RESULTS
    from concourse.bass_utils import run_bass_kernel_spmd

    hidden = np.ascontiguousarray(hidden, dtype=np.float32)
    encoder_outputs = np.ascontiguousarray(encoder_outputs, dtype=np.float32)
    attn_w = np.ascontiguousarray(attn_w, dtype=np.float32)
    attn_b = np.ascontiguousarray(attn_b, dtype=np.float32)
    v = np.ascontiguousarray(v, dtype=np.float32)
    assert hidden.shape == (B, D) and encoder_outputs.shape == (S, B, E)
    assert attn_w.shape == (D, E + D) and attn_b.shape == (D,) and v.shape == (D,)

    if _COMPILED is None:
        _COMPILED = _build()
    nc = _COMPILED

    in_maps = _prep_in_maps(hidden, encoder_outputs, attn_w, attn_b, v)
    res = run_bass_kernel_spmd(
        nc, in_maps, core_ids=list(range(N_CORES)),
        trace=PROFILE, **TRACE_KWARGS,
    )
    LAST_RESULTS = res
    # out [BL, 128, ST]: s = st*128 + p  ->  transpose to [BL, ST, 128]
    return np.concatenate(
        [res.results[c]["out"].transpose(0, 2, 1).reshape(BL, S)
         for c in range(N_CORES)], axis=0
    ).astype(np.float32)
